# revision 49
# baseline (speedup 1.0000x reference)
"""Multi-head attention (B=2, S=2048, D=1024, H=16) on 8 Trainium2 cores.

Sharding: core c handles batch b = c//4 and head group g = c%4 (4 heads,
256 of the 1024 QKV output columns).

v2 layout (vs baseline): the kernel emits UNNORMALIZED transposed
attention output per head — out_augT[dh+1, sq] where row 64 carries the
softmax denominator — straight from PSUM eviction to HBM. The host does
the final divide + transpose (cheap, and exact in fp32). This removes
all PE transposes, DVE reciprocals/normalizes and the serial end-tail.

Pipeline per (pair hp, sq-block j, sk-tile i):
  QK pair (2 heads on disjoint PE row groups, one 512-cycle slot)
  -> exp on ACT (scale folded into the activation affine)
  -> mask multiply on DVE (keepT in bf16 {0,1}; exact, exp(-1e9)==0)
  -> 2 PV matmuls accumulating [v|1].T @ expw into per-head PSUM.

Projections are emitted as a small prefix (just enough for the first
tiles) plus just-in-time filler chunks interleaved into the attention
stream, so the ACT exp stream starts ~10us in instead of ~40us.
DMA is issued in consumption order (weights, x halves, keepT i-chunks).
"""

import numpy as np

B, S, D, H = 2, 2048, 1024, 16
HD = D // H  # 64
HEADS_PER_CORE = 4
COLS = HEADS_PER_CORE * HD  # 256
N_CORES = 8
KT = D // 128  # 8 contraction tiles for projections
ST = S // 128  # 16 sk tiles
SCALE = 1.0 / np.sqrt(np.float32(D))

_cache = {}


def _build_nc():
    import concourse.bass as bass
    import concourse.mybir as mybir
    import concourse.tile as tile
    from concourse.bass import _add_dep_helper

    f32 = mybir.dt.float32
    bf16 = mybir.dt.bfloat16

    nc = bass.Bass(trn_type="TRN2")

    xT = nc.dram_tensor("xT", [D, S], bf16, kind="ExternalInput")
    wq = nc.dram_tensor("wq", [D, COLS], bf16, kind="ExternalInput")
    wk = nc.dram_tensor("wk", [D, COLS], bf16, kind="ExternalInput")
    wv = nc.dram_tensor("wv", [D, COLS], bf16, kind="ExternalInput")
    bq = nc.dram_tensor("bq", [128, 2], f32, kind="ExternalInput")
    bk = nc.dram_tensor("bk", [128, 2], f32, kind="ExternalInput")
    keepT = nc.dram_tensor("keepT", [S, S], bf16, kind="ExternalInput")
    # unnormalized transposed output: row h*65+d = head h dim d (d=64 is
    # the softmax denominator row)
    o = nc.dram_tensor("o", [HEADS_PER_CORE * (HD + 1), S], f32, kind="ExternalOutput")

    with tile.TileContext(nc) as tc:
        with (
            tc.tile_pool(name="singles", bufs=1) as singles,
            tc.tile_pool(name="persist", bufs=1) as persist,
            tc.tile_pool(name="big_ps", bufs=2, space="PSUM") as big_ps,
            tc.tile_pool(name="pv_ps", bufs=2, space="PSUM") as pv_ps,
            tc.tile_pool(name="tr_ps", bufs=2, space="PSUM") as tr_ps,
            tc.tile_pool(name="expw", bufs=14) as expw_pool,
            tc.tile_pool(name="expw2", bufs=4) as expw2_pool,
            tc.tile_pool(name="tails", bufs=2) as tails,
        ):
            # ---- constants / small inputs ----
            bq_sb = singles.tile([128, 2], f32)
            bk_sb = singles.tile([128, 2], f32)
            scratch = singles.tile([128, 640], bf16)
            nc.vector.memset(scratch, 0.0)

            # ---- bulk input SBUF tiles ----
            wq_sb = persist.tile([128, KT, COLS], bf16)
            wk_sb = persist.tile([128, KT, COLS], bf16)
            wv_sb = persist.tile([128, KT, COLS], bf16)
            xT_sb = persist.tile([128, KT, S], bf16)
            keepT_sb = persist.tile([128, ST, S], bf16)

            # Input DMA: consumption-ordered stream of ~0.5 MiB units,
            # stride-4 chained (unit n waits unit n-4). A single transfer
            # only sustains ~130 GB/s on its queue; 4 in flight reach the
            # ~400 GB/s aggregate while still arriving in order. The first
            # 4 units are the prefix-critical ones and run gate-free.
            xT_r = xT[:, :].rearrange("(kt p) s -> p kt s", p=128)
            keepT_r = keepT[:, :].rearrange("(i p) s -> p i s", p=128)

            gate = [None]
            units = {}

            def udma(name, out, in_):
                inst = nc.sync.dma_start(out=out, in_=in_)
                if gate[0] is not None:
                    _add_dep_helper(
                        inst.ins,
                        units[gate[0]].ins,
                        sync=True,
                        reason="input DMA stage order",
                    )
                units[name] = inst

            def keep_slice(j, ig):
                # mask(i, j) reads keepT_sb[:, i, j*512:(j+1)*512] — each
                # j-block consumes only a quarter of keepT's columns, so
                # stream it j-slice by j-slice (~0.5 MiB units of 4 sk
                # tiles each), matching the flat consumption order.
                udma(
                    f"kp{j}_{ig}",
                    keepT_sb[:, 4 * ig : 4 * ig + 4, j * 512 : (j + 1) * 512],
                    keepT_r[:, 4 * ig : 4 * ig + 4, j * 512 : (j + 1) * 512],
                )

            def x_half(sb, kh):
                udma(
                    f"x{sb}{kh}",
                    xT_sb[:, 4 * kh : 4 * kh + 4, sb * 512 : (sb + 1) * 512],
                    xT_r[:, 4 * kh : 4 * kh + 4, sb * 512 : (sb + 1) * 512],
                )

            # stage A — prefix-critical (biases are tiny; the QK-chain
            # evictions need them right after the chains)
            x_half(0, 0)
            x_half(0, 1)
            udma("wk", wk_sb, wk[:, :].rearrange("(kt p) c -> p kt c", p=128))
            udma("wq", wq_sb, wq[:, :].rearrange("(kt p) c -> p kt c", p=128))
            udma("bq", bq_sb, bq[:, :])
            udma("bk", bk_sb, bk[:, :])
            # stage B
            gate[0] = "wq"
            udma("wv", wv_sb, wv[:, :].rearrange("(kt p) c -> p kt c", p=128))
            x_half(1, 0)
            x_half(1, 1)
            keep_slice(0, 0)
            # stage C
            gate[0] = "x10"
            x_half(2, 0)
            x_half(2, 1)
            keep_slice(0, 1)
            keep_slice(0, 2)
            # stage D
            gate[0] = "x20"
            x_half(3, 0)
            x_half(3, 1)
            keep_slice(0, 3)
            keep_slice(1, 0)
            # stage E
            gate[0] = "x30"
            keep_slice(1, 1)
            keep_slice(1, 2)
            keep_slice(1, 3)
            keep_slice(2, 0)
            # stage F
            gate[0] = "kp1_1"
            keep_slice(2, 1)
            keep_slice(2, 2)
            keep_slice(2, 3)
            keep_slice(3, 0)
            keep_slice(3, 1)
            keep_slice(3, 2)
            keep_slice(3, 3)
            gate[0] = None

            # ---- projection building blocks ----
            # qT/kT: [128 (2 heads of dh), blk, s]; head h lives at
            # partitions (h%2)*64.. of block h//2.
            qT_sb = persist.tile([128, 2, S], bf16)
            kT_sb = persist.tile([128, 2, S], bf16)

            qk_chain_ps = {}

            def project_qk_half(which, blk, jh, nn, part):
                """Half of a [128,512] psum chain: part 0 = first 4 kt
                matmuls, part 1 = last 4 + bias-add eviction. Both parts
                share one psum tile (stashed across filler slots)."""
                w_sb, b_sb, dst = (
                    (wq_sb, bq_sb, qT_sb),
                    (wk_sb, bk_sb, kT_sb),
                )[which]
                key = (which, blk, jh, nn)
                if part == 0:
                    ps = tr_ps.tile([128, 512], f32, tag="tr")
                    qk_chain_ps[key] = ps
                else:
                    ps = qk_chain_ps.pop(key)
                for kt in (range(0, 4) if part == 0 else range(4, KT)):
                    nc.tensor.matmul(
                        ps,
                        lhsT=w_sb[:, kt, blk * 128 : (blk + 1) * 128],
                        rhs=xT_sb[
                            :, kt, jh * 1024 + nn * 512 : jh * 1024 + (nn + 1) * 512
                        ],
                        start=(kt == 0),
                        stop=(kt == KT - 1),
                        skip_group_check=True,
                    )
                if part == 1:
                    nc.vector.tensor_scalar_add(
                        out=dst[
                            :, blk, jh * 1024 + nn * 512 : jh * 1024 + (nn + 1) * 512
                        ],
                        in0=ps,
                        scalar1=b_sb[:, blk : blk + 1],
                    )

            # v in natural layout, augmented with a ones column per head:
            # v_aug[p, st, h, 0:64] = v, v_aug[p, st, h, 64] = 1
            v_aug = persist.tile([128, ST, HEADS_PER_CORE, HD + 1], bf16)
            nc.vector.memset(v_aug[:, :, :, HD : HD + 1], 1.0)

            def project_v(st):
                # no bias: out = num/den + bv holds exactly, so bv is
                # added on the host after normalization (rank-1 identity
                # via the denominator row).
                psv = tr_ps.tile([128, COLS], f32, tag="tr", name=f"v{st}")
                for kt in range(KT):
                    nc.tensor.matmul(
                        psv,
                        lhsT=xT_sb[:, kt, st * 128 : (st + 1) * 128],
                        rhs=wv_sb[:, kt, :],
                        start=(kt == 0),
                        stop=(kt == KT - 1),
                        skip_group_check=True,
                    )
                nc.vector.tensor_copy(
                    out=v_aug[:, st, :, 0:HD],
                    in_=psv.rearrange("p (h d) -> p h d", h=HEADS_PER_CORE),
                )

            # ---- filler schedule ----
            # Per-tile lists of projection chunks emitted inside the
            # attention loop (tile index t = hp*64 + j*16 + i). Deadlines:
            #   v_st(k)      before PV of tile t=k (emit by t=k-1)
            #   k b0 nn1     before QK t=4;   k b0 jh1 before t=8/t=12
            #   q b0 nn1     before t=16 (j=1); q b0 jh1 before t=32/48
            #   k/q b1 *     before t=64 (+4 per i-tile, +16 per j)
            def V(k):
                return lambda: project_v(k)

            def QK(w, blk, jh, nn, p):
                return lambda: project_qk_half(w, blk, jh, nn, p)

            fill_sched = {
                # one chunk per tile; tiles 0-1 run filler-free so the exp
                # stream launches on xA+weights alone. All deadlines have
                # LAG of extra slack for the v chunks.
                2: [QK(1, 0, 0, 1, 0)],
                3: [QK(1, 0, 0, 1, 1)],
                4: [V(0)],
                5: [QK(1, 0, 1, 0, 0)],
                6: [QK(1, 0, 1, 0, 1)],
                7: [QK(1, 0, 1, 1, 0)],
                8: [QK(1, 0, 1, 1, 1)],
                9: [V(1)],
                10: [V(2)],
                11: [V(3)],
                12: [V(4)],
                13: [QK(0, 0, 0, 1, 0)],
                14: [V(5)],
                15: [QK(0, 0, 0, 1, 1)],
                16: [V(6)],
                17: [V(7)],
                18: [V(8)],
                19: [V(9)],
                20: [V(10)],
                21: [V(11)],
                22: [V(12)],
                23: [V(13)],
                24: [V(14)],
                25: [V(15)],
                # spread chunks, each well before its deadline
                27: [QK(0, 0, 1, 0, 0)],
                29: [QK(0, 0, 1, 0, 1)],
                33: [QK(0, 0, 1, 1, 0)],
                37: [QK(0, 0, 1, 1, 1)],
                41: [QK(1, 1, 0, 0, 0)],
                45: [QK(1, 1, 0, 0, 1)],
                49: [QK(0, 1, 0, 0, 0)],
                53: [QK(0, 1, 0, 0, 1)],
                57: [QK(1, 1, 0, 1, 0)],
                60: [QK(1, 1, 0, 1, 1)],
                63: [QK(1, 1, 1, 0, 0)],
                66: [QK(1, 1, 1, 0, 1)],
                69: [QK(1, 1, 1, 1, 0)],
                72: [QK(1, 1, 1, 1, 1)],
                75: [QK(0, 1, 0, 1, 0)],
                78: [QK(0, 1, 0, 1, 1)],
                83: [QK(0, 1, 1, 0, 0)],
                88: [QK(0, 1, 1, 0, 1)],
                94: [QK(0, 1, 1, 1, 0)],
                100: [QK(0, 1, 1, 1, 1)],
            }

            # ---- flat attention loop with lagged mask/PV ----
            # QK+exp for flat tile t are emitted at t; the mask+PV (and,
            # at block ends, the psum eviction + output DMA) for tile t-L
            # are emitted at t. The lag keeps the in-order PE/DVE streams
            # from stalling on keepT DMA arrival early on, and moves each
            # sq-block's tail off the next block's critical path.
            LAG = 12
            pend = {}
            pvs_by_block = {}

            def emit_qk_exp(t):
                hp, j, i = t // 64, (t % 64) // 16, t % 16
                lgp = big_ps.tile([128, 1024], f32, tag="big")
                for e in range(2):
                    po = e * 64
                    nc.tensor.matmul(
                        lgp[:, e * 512 : (e + 1) * 512],
                        lhsT=kT_sb[po : po + 64, hp, i * 128 : (i + 1) * 128],
                        rhs=qT_sb[po : po + 64, hp, j * 512 : (j + 1) * 512],
                        start=True,
                        stop=True,
                        skip_group_check=True,
                    )
                ex = expw_pool.tile([128, 1024], bf16)
                nc.scalar.activation(
                    out=ex,
                    in_=lgp,
                    func=mybir.ActivationFunctionType.Exp,
                    scale=float(SCALE),
                )
                pend[t] = ex

            def emit_mask_pv(t):
                hp, j, i = t // 64, (t % 64) // 16, t % 16
                ex = pend.pop(t)
                # mask: multiply both heads' halves by the same keepT
                # slice, read twice via a stride-0 broadcast dim
                ex2 = expw2_pool.tile([128, 1024], bf16)
                k_ap = keepT_sb[:, i, j * 512 : (j + 1) * 512]
                k_bcast = bass.AP(
                    tensor=k_ap.tensor,
                    offset=k_ap.offset,
                    ap=[k_ap.ap[0], [0, 2], *k_ap.ap[1:]],
                )
                nc.vector.tensor_mul(
                    out=ex2.rearrange("p (e n) -> p e n", e=2),
                    in0=ex.rearrange("p (e n) -> p e n", e=2),
                    in1=k_bcast,
                )
                if i == 0:
                    pvs_by_block[(hp, j)] = [
                        pv_ps.tile([HD + 1, 512], f32, tag="pv", name=f"pv{e}")
                        for e in range(2)
                    ]
                pvs = pvs_by_block[(hp, j)]
                for e in range(2):
                    nc.tensor.matmul(
                        pvs[e],
                        lhsT=v_aug[:, i, 2 * hp + e, :],
                        rhs=ex2[:, e * 512 : (e + 1) * 512],
                        start=(i == 0),
                        stop=(i == ST - 1),
                        skip_group_check=True,
                    )
                if i == ST - 1:
                    # tail: evict both heads' unnormalized [65, 512] slabs
                    # to SBUF, then one DMA to the transposed HBM output.
                    pvs = pvs_by_block.pop((hp, j))
                    pv_sb = tails.tile([HD + 1, 2, 512], f32, tag="pvsb")
                    for e in range(2):
                        nc.vector.tensor_copy(out=pv_sb[:, e, :], in_=pvs[e])
                    nc.sync.dma_start(
                        out=o[
                            2 * hp * (HD + 1) : (2 * hp + 2) * (HD + 1),
                            j * 512 : (j + 1) * 512,
                        ].rearrange("(e p) s -> p e s", p=HD + 1),
                        in_=pv_sb,
                    )

            # ---- emission: minimal prefix, then the flat loop ----
            # Warm the PE's HAM clock-gate on scratch data while the first
            # DMAs are in flight, so the prefix chains run at 2.4 GHz.
            # ~18 dummy matmuls bridge the xA/wk DMA wait (~8..15.5us) so
            # the HAM stays continuously busy and the real chains run at
            # 2.4 GHz. (8 cold MMs warm it; the rest keep it warm.)
            warm_ps = tr_ps.tile([128, 512], f32, tag="tr")
            for w in range(18):
                nc.tensor.matmul(
                    warm_ps,
                    lhsT=scratch[:, 0:128],
                    rhs=scratch[:, 128:640],
                    start=(w == 0),
                    stop=(w == 17),
                    skip_group_check=True,
                )
            project_qk_half(1, 0, 0, 0, 0)  # k blk0 jh0 nn0
            project_qk_half(1, 0, 0, 0, 1)
            project_qk_half(0, 0, 0, 0, 0)  # q blk0 jh0 nn0
            project_qk_half(0, 0, 0, 0, 1)

            # lag tapers from LAG to 2 over t=96..116 (keepT is resident
            # by then) so only 2 mask/PV slots remain after the last exp.
            def lag_at(t):
                if t < 96:
                    return LAG
                return max(2, LAG - (t - 95) // 2)

            mp = 0  # next tile to mask/PV
            for t in range(64 * 2 + 2):
                if t < 128:
                    emit_qk_exp(t)
                    for fn in fill_sched.pop(t, ()):
                        fn()
                target = 127 if t >= 128 else t - lag_at(t)
                while mp <= target:
                    emit_mask_pv(mp)
                    mp += 1

    # Workaround: this container's walrus encodes at most one sync wait per
    # instruction — split multi-wait instructions into single-wait NoOps.
    _split_multiwait(nc)
    return nc


def _split_multiwait(nc, max_waits: int = 1):
    import concourse.mybir as mybir

    for f in nc.m.functions:
        for blk in f.blocks:
            out = []
            changed = False
            for inst in blk.instructions:
                si = inst.sync_info
                if si is not None and len(si.on_wait) > max_waits:
                    waits = list(si.on_wait)
                    extra = waits[: len(waits) - max_waits]
                    keep = waits[len(waits) - max_waits :]
                    for k, w in enumerate(extra):
                        out.append(
                            mybir.InstNoOp(
                                name=f"{inst.name}-wfx{k}",
                                engine=inst.engine,
                                sync_info=mybir.SyncInfo(on_wait=[w], on_update=[]),
                                bass_nofuse=True,
                            )
                        )
                    inst.sync_info = mybir.SyncInfo(
                        on_wait=keep, on_update=list(si.on_update)
                    )
                    changed = True
                out.append(inst)
            if changed:
                blk.instructions = out


def _prep_in_maps(x, mask, Wq, bq, Wk, bk, Wv, bv):
    import ml_dtypes

    bf16 = ml_dtypes.bfloat16
    x = np.asarray(x, np.float32)
    mask = np.asarray(mask, bool)

    xT_b = [np.ascontiguousarray(x[b].T).astype(bf16) for b in range(B)]
    keepT_b = [
        np.ascontiguousarray((~mask[b, 0]).T).astype(bf16) for b in range(B)
    ]
    WqT = np.asarray(Wq, np.float32).T.astype(bf16)
    WkT = np.asarray(Wk, np.float32).T.astype(bf16)
    WvT = np.asarray(Wv, np.float32).T.astype(bf16)
    bq32 = np.asarray(bq, np.float32)
    bk32 = np.asarray(bk, np.float32)

    in_maps = []
    for c in range(N_CORES):
        b, g = divmod(c, 4)
        cols = slice(g * COLS, (g + 1) * COLS)
        in_maps.append(
            {
                "xT": xT_b[b],
                "wq": np.ascontiguousarray(WqT[:, cols]),
                "wk": np.ascontiguousarray(WkT[:, cols]),
                "wv": np.ascontiguousarray(WvT[:, cols]),
                "bq": np.ascontiguousarray(bq32[cols].reshape(2, 128).T),
                "bk": np.ascontiguousarray(bk32[cols].reshape(2, 128).T),
                "keepT": keepT_b[b],
            }
        )
    return in_maps


def kernel(x, mask, Wq, bq, Wk, bk, Wv, bv, _trace=False):
    from concourse.bass_utils import run_bass_kernel_spmd

    if "nc" not in _cache:
        _cache["nc"] = _build_nc()
    nc = _cache["nc"]

    in_maps = _prep_in_maps(x, mask, Wq, bq, Wk, bk, Wv, bv)
    res = run_bass_kernel_spmd(
        nc, in_maps, core_ids=list(range(N_CORES)), trace=_trace
    )
    _cache["last_result"] = res

    bv32 = np.asarray(bv, np.float32)
    out = np.empty((B, S, D), np.float32)
    for c in range(N_CORES):
        b, g = divmod(c, 4)
        oT = res.results[c]["o"].reshape(HEADS_PER_CORE, HD + 1, S)
        num = oT[:, 0:HD, :]  # [4, 64, S]
        den = oT[:, HD : HD + 1, :]  # [4, 1, S]
        res_c = (num / den).transpose(2, 0, 1).reshape(S, COLS)
        out[b, :, g * COLS : (g + 1) * COLS] = res_c + bv32[g * COLS : (g + 1) * COLS]
    return out
